# revision 11
# baseline (speedup 1.0000x reference)
"""TRN2 Bass kernel for nn_Decoder_SICA (dense CNN decoder), 8-core data parallel.

v2: conv1 via 2D Winograd F(2x2, 3x3) + all-fp16 matmuls.

Network (per sample):
  stage0: per-sample grouped conv_transpose (stride==kernel) == block einsum
          A(512,2,2) x S(64,3,5,5) -> x0 (384,10,10)
  conv1:  384->384 3x3 pad1        -> (384,10,10) relu   [Winograd F(2,3)]
  conv2:  384->384 3x3 s2 pad1     -> (384,5,5)   relu
  conv3:  384->512 3x3 pad1        -> (512,5,5)   relu
  conv4:  512->512 5x5 valid       -> (512,1,1)   relu
  linear: 512->10

Design notes:
  - batch sharded 8 ways (64/core); all matmuls fp16 (PSUM fp32).
  - conv1 Winograd F(2,3): 25 2x2-output tiles on the zero-padded 12x12
    input, batch slabs of 8.  DVE computes B^T d B as
    scalar_tensor_tensor ops (<=2 free dims, 4x 16-bit mode); PE contracts
    V with host-precomputed U = G W1 G^T per k-point (16 k, N=200);
    Act+Pool evict M from PSUM to SBUF fp16; DVE folds A^T(.)  (tmpo),
    Pool folds (.)A (y), DVE applies fused bias+ReLU into x1.
    Tensor rows: 230k vs 452k direct.
  - column-parity layouts (x0pad/tmpq split cols into (par, cp); x1
    batch kept slab-major) keep every DVE/Pool op at <=2 merged free
    dims, which the ScalarTensorTensor ISA requires.
  - PSUM (8 banks): 2x conv1 a-tiles [P,4,256]f32 (2 banks each) +
    2 stage0 + 2 conv2/3/4.
  - stage0 of slab s+1 and the input transform of slab s+1 are emitted
    between conv1 cot-blocks of slab s so Act/DVE overlap PE.
"""

import numpy as np
import ml_dtypes

import concourse.bacc as bacc
import concourse.mybir as mybir
from concourse.tile import TileContext
from concourse.bass_utils import run_bass_kernel_spmd

P = 128
B_FULL = 512
NCORES = 8
BL = B_FULL // NCORES        # 64 samples per core
BS = 8                       # winograd batch slab
NSLAB = BL // BS
F32R = mybir.dt.float32r
FP16 = mybir.dt.float16
FP32 = mybir.dt.float32

# B^T row/col combinations: out[j] = d[s0] + sgn*d[s1]
SEL = [(0, 2, -1.0), (1, 2, 1.0), (2, 1, -1.0), (1, 3, -1.0)]


def _build_program(loop_n=1, accum_out=False):
    nc = bacc.Bacc("TRN2", target_bir_lowering=False, debug=False,
                   num_devices=NCORES)

    # ---- DRAM I/O (per core) ----
    abd = nc.dram_tensor("abd", [BL // 4, 64, 4, 4 * P], FP16, kind="ExternalInput")
    s2 = nc.dram_tensor("s2", [BL // 4, 64, 4, 75], FP16, kind="ExternalInput")
    u1 = nc.dram_tensor("u1", [3, P, 3, 16, P], FP16, kind="ExternalInput")
    w2t = nc.dram_tensor("w2t", [3, P, 3, 3, 3, P], FP16, kind="ExternalInput")
    w3t = nc.dram_tensor("w3t", [4, P, 3, 3, 3, P], FP16, kind="ExternalInput")
    w4t = nc.dram_tensor("w4t", [4, P, 4, 5, 5, P], FP16, kind="ExternalInput")
    wl = nc.dram_tensor("wl", [P, 4, 10], FP16, kind="ExternalInput")
    biases = nc.dram_tensor("biases", [P, 14], FP32, kind="ExternalInput")
    blrep = nc.dram_tensor("blrep", [BL, 10], FP32, kind="ExternalInput")
    out_d = nc.dram_tensor("OUT", [BL, 10], FP32, kind="ExternalOutput")
    # bias columns: b1 -> 0:3, b2 -> 3:6, b3 -> 6:10, b4 -> 10:14

    RELU = mybir.ActivationFunctionType.Relu

    with TileContext(nc) as tc:
        with (
            tc.tile_pool(name="persist", bufs=1) as persist,
            tc.tile_pool(name="vpool", bufs=2) as vpool,       # V double buffer
            tc.tile_pool(name="mpool", bufs=2) as mpool,       # msb/tmpo/y_pre
            tc.tile_pool(name="wpool", bufs=3) as wpool,       # w2/w3/w4 stream
            tc.tile_pool(name="s0pool", bufs=3) as s0pool,     # stage0 staging
            tc.tile_pool(name="psA", bufs=2, space="PSUM") as psA,
            tc.tile_pool(name="ps0pool", bufs=2, space="PSUM") as ps0pool,
            tc.tile_pool(name="pspool", bufs=2, space="PSUM") as pspool,
        ):
            bias_t = persist.tile([P, 14], FP32, name="bias_t")
            nc.sync.dma_start(bias_t[:], biases[:])
            wl_t = persist.tile([P, 4, 10], FP16, name="wl_t")
            nc.sync.dma_start(wl_t[:], wl[:])
            bl_t = persist.tile([BL, 10], FP32, name="bl_t")
            nc.sync.dma_start(bl_t[:], blrep[:])
            u1_t = persist.tile([P, 3, 3, 16, P], FP16, name="u1_t")
            for cot in range(3):
                nc.sync.dma_start(u1_t[:, cot], u1[cot])

            # x0pad: [cit, row, colpar, colhalf, b]; halo zeroed once
            x0pad = persist.tile([P, 3, 12, 2, 6, BS], FP16, name="x0pad")
            nc.gpsimd.memset(x0pad[:, :, 0], 0.0)
            nc.gpsimd.memset(x0pad[:, :, 11], 0.0)
            nc.gpsimd.memset(x0pad[:, :, 1:11, 0, 0, :], 0.0)   # col 0
            nc.gpsimd.memset(x0pad[:, :, 1:11, 1, 5, :], 0.0)   # col 11

            tmpq = persist.tile([P, 3, 5, 2, 6, BS], FP16, name="tmpq")
            # x1: slab-major batch [cit, s, row, colpar, colhalf, bi]
            x1 = persist.tile([P, 3, NSLAB, 10, 2, 5, BS], FP16, name="x1")
            x2 = persist.tile([P, 3, 5, 5, BL], FP16, name="x2")
            x3 = persist.tile([P, 4, 5, 5, BL], FP16, name="x3")
            x4 = persist.tile([P, 4, BL], FP16, name="x4")

            def body():
                _emit_body(nc, tc, persist, vpool, mpool, wpool, s0pool,
                           psA, ps0pool, pspool, bias_t, wl_t, bl_t,
                           x0pad, tmpq, x1, x2, x3, x4, u1_t,
                           abd, s2, w2t, w3t, w4t, out_d, RELU, accum_out)

            if loop_n > 1:
                with tc.For_i(0, loop_n, 1):
                    body()
            else:
                body()

    nc.compile()
    return nc


def _stage0_group(nc, s0pool, ps0pool, abd, s2, x0pad, g, bslot0):
    """stage0 for one group of 4 samples -> x0pad interior slots."""
    st = s0pool.tile([64, 4, 4 * P + 75], FP16, name=f"st{g}", tag="st")
    nc.sync.dma_start(st[:, :, : 4 * P], abd[g])
    nc.sync.dma_start(st[:, :, 4 * P:], s2[g])
    for bi in range(4):
        ps0 = ps0pool.tile([P, 3, 10, 10], FP32, name=f"ps0_{g}_{bi}",
                           tag="ps0")
        for hw in range(4):
            h, w = hw // 2, hw % 2
            nc.tensor.matmul(
                ps0[:, :, 5 * h: 5 * h + 5, 5 * w: 5 * w + 5],
                lhsT=st[:, bi, hw * P: (hw + 1) * P],
                rhs=st[:, bi, 4 * P:],
                start=(hw == 0), stop=(hw == 3),
            )
        b = bslot0 + bi
        # padded col c holds x0 col c-1; parity-split dst needs 2 copies:
        # even cols 2,4,..,10 -> (par0, cp 1..5) from x0 cols 1,3,..,9
        nc.scalar.copy(x0pad[:, :, 1:11, 0, 1:6, b], ps0[:, :, :, 1:10:2])
        # odd cols 1,3,..,9 -> (par1, cp 0..4) from x0 cols 0,2,..,8
        nc.scalar.copy(x0pad[:, :, 1:11, 1, 0:5, b], ps0[:, :, :, 0:9:2])


def _emit_body(nc, tc, persist, vpool, mpool, wpool, s0pool, psA, ps0pool,
               pspool, bias_t, wl_t, bl_t, x0pad, tmpq, x1, x2, x3, x4,
               u1_t, abd, s2, w2t, w3t, w4t, out_d, RELU, accum_out=False):
    AL = mybir.AluOpType
    STV = nc.vector.scalar_tensor_tensor

    def aop(sgn):
        return AL.add if sgn > 0 else AL.subtract

    def input_transform(s):
        """DVE: x0pad -> V (B^T d B) for slab s. Returns the V tile."""
        v_t = vpool.tile([P, 16, 3, 25, BS], FP16, name=f"v{s}", tag="V")
        for a, (r0, r1, sgn) in enumerate(SEL):
            # tmpq[cit, th, cpar, cp, b] = xp[2th+r0, col] +- xp[2th+r1, col]
            for cit in range(3):
                STV(tmpq[:, cit], x0pad[:, cit, r0:r0 + 9:2], 1.0,
                    x0pad[:, cit, r1:r1 + 9:2], op0=AL.mult, op1=aop(sgn))
            for i, (c0, c1, sg2) in enumerate(SEL):
                # V[a*4+i][cit, (th,tw), b] = tmpq[.., col 2tw+c0]
                #                             +- tmpq[.., col 2tw+c1]
                STV(v_t[:, a * 4 + i],
                    tmpq[:, :, :, c0 % 2, c0 // 2: c0 // 2 + 5, :], 1.0,
                    tmpq[:, :, :, c1 % 2, c1 // 2: c1 // 2 + 5, :],
                    op0=AL.mult, op1=aop(sg2))
        return v_t

    def conv1_cot(s, cot, v_t):
        """PE matmuls + M eviction + output transform for one cot."""
        msb = mpool.tile([P, 4, 4, 25, BS], FP16, name=f"m{s}_{cot}",
                         tag="msb")
        tmpo = mpool.tile([P, 2, 4, 25, BS], FP16, name=f"t{s}_{cot}",
                          tag="tmpo")
        y_pre = mpool.tile([P, 2, 2, 5, 5, BS], FP16, name=f"y{s}_{cot}",
                           tag="ypre")
        for a in range(4):
            pa = psA.tile([P, 4, 256], FP32, name=f"pa{s}_{cot}_{a}",
                          tag="pa")
            for i in range(4):
                k = a * 4 + i
                for cit in range(3):
                    nc.tensor.matmul(
                        pa[:, i, :200],
                        lhsT=u1_t[:, cot, cit, k, :],
                        rhs=v_t[:, k, cit],
                        start=(cit == 0), stop=(cit == 2),
                        skip_group_check=True,
                    )
            # evict M (4 k-planes) to SBUF fp16 (GPSIMD cannot read PSUM)
            nc.scalar.copy(msb[:, a], pa[:, :, :200])
        # tmpo[p] = A^T M (over a):  p0 = m0+m1+m2 ; p1 = m1-m2-m3  (DVE)
        STV(tmpo[:, 0], msb[:, 0], 1.0, msb[:, 1], op0=AL.mult, op1=AL.add)
        STV(tmpo[:, 0], tmpo[:, 0], 1.0, msb[:, 2], op0=AL.mult, op1=AL.add)
        STV(tmpo[:, 1], msb[:, 1], 1.0, msb[:, 2], op0=AL.mult,
            op1=AL.subtract)
        STV(tmpo[:, 1], tmpo[:, 1], 1.0, msb[:, 3], op0=AL.mult,
            op1=AL.subtract)
        # y_pre[p, q] = (tmpo A)[q over i]  (Pool; TensorTensor only there)
        for p in range(2):
            nc.gpsimd.tensor_add(y_pre[:, p, 0], tmpo[:, p, 0], tmpo[:, p, 1])
            nc.gpsimd.tensor_add(y_pre[:, p, 0], y_pre[:, p, 0], tmpo[:, p, 2])
            nc.gpsimd.tensor_sub(y_pre[:, p, 1], tmpo[:, p, 1], tmpo[:, p, 2])
            nc.gpsimd.tensor_sub(y_pre[:, p, 1], y_pre[:, p, 1], tmpo[:, p, 3])
        # fused bias + relu -> x1 rows (2th+p), cols (par q, cp tw)  (DVE)
        for p in range(2):
            for q in range(2):
                nc.vector.tensor_scalar(
                    x1[:, cot, s, p:p + 9:2, q], y_pre[:, p, q],
                    bias_t[:, cot:cot + 1], 0.0, op0=AL.add, op1=AL.max)

    # ---------------- stage0 + conv1 pipeline ----------------
    _stage0_group(nc, s0pool, ps0pool, abd, s2, x0pad, 0, 0)
    _stage0_group(nc, s0pool, ps0pool, abd, s2, x0pad, 1, 4)
    v_cur = input_transform(0)
    for s in range(NSLAB):
        if s + 1 < NSLAB:
            _stage0_group(nc, s0pool, ps0pool, abd, s2, x0pad,
                          2 * (s + 1), 0)
        conv1_cot(s, 0, v_cur)
        if s + 1 < NSLAB:
            _stage0_group(nc, s0pool, ps0pool, abd, s2, x0pad,
                          2 * (s + 1) + 1, 4)
        conv1_cot(s, 1, v_cur)
        if s + 1 < NSLAB:
            v_next = input_transform(s + 1)
        conv1_cot(s, 2, v_cur)
        if s + 1 < NSLAB:
            v_cur = v_next

    # ---------------- conv2: 384->384 3x3 stride2 pad1 ----------------
    # x1 col j stored at (par=j%2, cp=j//2); stride-2 taps share parity.
    for cot in range(3):
        w_t = wpool.tile([P, 3, 3, 3, P], FP16, name=f"w2_{cot}", tag="w")
        nc.sync.dma_start(w_t[:], w2t[cot])
        for r in range(5):
            pt = pspool.tile([P, 5, BL], FP32, name=f"p2_{cot}_{r}", tag="ps")
            first = True
            for cit in range(3):
                for dh in (-1, 0, 1):
                    ir = 2 * r + dh
                    if ir < 0 or ir > 9:
                        continue
                    for dw in (-1, 0, 1):
                        ow0 = 1 if dw == -1 else 0
                        nw = 5 - ow0
                        iw0 = 2 * ow0 + dw   # x1 col of first tap
                        par, cp = iw0 % 2, iw0 // 2
                        rhs = x1[:, cit, :, ir, par, cp:cp + nw, :]
                        rhs = rhs.rearrange("p s w b -> p w s b")
                        nc.tensor.matmul(
                            pt[:, ow0:5, :],
                            lhsT=w_t[:, cit, dh + 1, dw + 1, :],
                            rhs=rhs,
                            start=first, stop=False,
                            skip_group_check=True,
                        )
                        first = False
            nc.scalar.activation(
                x2[:, cot, r, :, :], pt[:], RELU,
                bias=bias_t[:, 3 + cot: 4 + cot],
            )

    # ---------------- conv3: 384->512 3x3 pad1 ----------------
    for cot in range(4):
        w_t = wpool.tile([P, 3, 3, 3, P], FP16, name=f"w3_{cot}", tag="w")
        nc.sync.dma_start(w_t[:], w3t[cot])
        for r in range(5):
            pt = pspool.tile([P, 5, BL], FP32, name=f"p3_{cot}_{r}", tag="ps")
            first = True
            for cit in range(3):
                for dh in (-1, 0, 1):
                    ir = r + dh
                    if ir < 0 or ir > 4:
                        continue
                    for dw in (-1, 0, 1):
                        ow0, own = max(0, -dw), min(5, 5 - dw)
                        iw0 = ow0 + dw
                        nw = own - ow0
                        nc.tensor.matmul(
                            pt[:, ow0:own, :],
                            lhsT=w_t[:, cit, dh + 1, dw + 1, :],
                            rhs=x2[:, cit, ir, iw0: iw0 + nw, :],
                            start=first, stop=False,
                            skip_group_check=True,
                        )
                        first = False
            nc.scalar.activation(
                x3[:, cot, r, :, :], pt[:], RELU,
                bias=bias_t[:, 6 + cot: 7 + cot],
            )

    # ---------------- conv4: 512->512 5x5 valid ----------------
    for cot in range(4):
        pt = pspool.tile([P, BL], FP32, name=f"p4_{cot}", tag="ps")
        first = True
        for cit in range(4):
            w_t = wpool.tile([P, 5, 5, P], FP16, name=f"w4_{cot}_{cit}",
                             tag="w")
            nc.sync.dma_start(w_t[:], w4t[cot, :, cit])
            for rr in range(5):
                for cc in range(5):
                    nc.tensor.matmul(
                        pt[:],
                        lhsT=w_t[:, rr, cc, :],
                        rhs=x3[:, cit, rr, cc, :],
                        start=first,
                        stop=(cit == 3 and rr == 4 and cc == 4),
                    )
                    first = False
        nc.scalar.activation(
            x4[:, cot, :], pt[:], RELU,
            bias=bias_t[:, 10 + cot: 11 + cot],
        )

    # ---------------- linear: 512 -> 10 ----------------
    pl = pspool.tile([BL, 10], FP32, name="pl", tag="ps")
    for cit in range(4):
        nc.tensor.matmul(
            pl[:], lhsT=x4[:, cit, :], rhs=wl_t[:, cit, :],
            start=(cit == 0), stop=(cit == 3),
        )
    out_sb = persist.tile([BL, 10], FP32, name="out_sb")
    nc.vector.tensor_add(out_sb[:], pl[:], bl_t[:])
    if accum_out:
        nc.gpsimd.dma_start(out_d[:], out_sb[:], accum_op=mybir.AluOpType.add)
    else:
        nc.sync.dma_start(out_d[:], out_sb[:])


def _prep_core_inputs(A, S, W1, b1, W2, b2, W3, b3, W4, b4, Wl, bl):
    """Host-side reshapes (pure data movement). Returns list of per-core dicts."""
    B = A.shape[0]
    # stage0 block-diagonal stationary from A:
    # abd[b, hw, g*4+c, n*16+g] = A[b, n*64+g*4+c, h, w]
    A_r = np.ascontiguousarray(
        A.reshape(B, 8, 16, 4, 4).transpose(0, 4, 2, 3, 1)
    )  # [B, hw, g, c, n]
    abd = np.zeros((B, 4, 64, P), dtype=np.float16)
    abd_v = abd.reshape(B, 4, 16, 4, 8, 16)  # [B, hw, g, c, n, g']
    gg = np.arange(16)
    abd_v[:, :, gg, :, :, gg] = A_r.transpose(2, 0, 1, 3, 4)[gg].astype(
        np.float16
    )
    # group-major staging layout: [B/4, part, b_in_group, hw*128]
    abd = np.ascontiguousarray(
        abd.reshape(B // 4, 4, 4, 64, P).transpose(0, 3, 1, 2, 4)
    ).reshape(B // 4, 64, 4, 4 * P)
    s2 = S.reshape(B, 64, 75).astype(np.float16)
    s2 = np.ascontiguousarray(
        s2.reshape(B // 4, 4, 64, 75).transpose(0, 2, 1, 3)
    )

    # x0's channel layout from stage0 is (o, n*16+g); permute W1's ci to match
    # (reference ci index = n*48 + g*3 + o).
    o_i, n_i, g_i = np.meshgrid(
        np.arange(3), np.arange(8), np.arange(16), indexing="ij"
    )
    perm = (n_i * 48 + g_i * 3 + o_i).reshape(-1)
    W1p = W1[:, perm].astype(np.float32)
    G = np.array([[1, 0, 0], [.5, .5, .5], [.5, -.5, .5], [0, 0, 1]],
                 np.float32)
    # U4[o, a, l, c] = G W G^T  (a = row-transform idx, l = col idx)
    U4 = np.einsum("aj,ocjk,lk->oalc", G, W1p, G)
    u1 = np.ascontiguousarray(
        U4.reshape(3, P, 4, 4, 3, P).transpose(0, 5, 4, 2, 3, 1)
        .reshape(3, P, 3, 16, P)
    ).astype(np.float16)

    def conv_w_tiles(W, n_cot, n_cit, k):
        t = W.reshape(n_cot, P, n_cit, P, k, k).transpose(0, 3, 2, 4, 5, 1)
        return np.ascontiguousarray(t).astype(np.float16)

    w2t = conv_w_tiles(W2, 3, 3, 3)
    w3t = conv_w_tiles(W3, 4, 3, 3)
    w4t = conv_w_tiles(W4, 4, 4, 5)

    wl_a = np.ascontiguousarray(
        Wl.T.reshape(4, P, 10).transpose(1, 0, 2)
    ).astype(np.float16)
    biases = np.zeros((P, 14), np.float32)
    biases[:, 0:3] = b1.reshape(3, P).T
    biases[:, 3:6] = b2.reshape(3, P).T
    biases[:, 6:10] = b3.reshape(4, P).T
    biases[:, 10:14] = b4.reshape(4, P).T
    blrep = np.tile(bl.astype(np.float32), (BL, 1))

    in_maps = []
    for c in range(NCORES):
        sl = slice(c * BL // 4, (c + 1) * BL // 4)
        in_maps.append({
            "abd": abd[sl], "s2": s2[sl], "u1": u1,
            "w2t": w2t, "w3t": w3t, "w4t": w4t,
            "wl": wl_a, "biases": biases, "blrep": blrep,
        })
    return in_maps


_PROGRAM_CACHE = {}


def _get_program():
    if "nc" not in _PROGRAM_CACHE:
        _PROGRAM_CACHE["nc"] = _build_program()
    return _PROGRAM_CACHE["nc"]


def kernel(A, S, W1, b1, W2, b2, W3, b3, W4, b4, Wl, bl):
    A = np.asarray(A, np.float32)
    S = np.asarray(S, np.float32)
    in_maps = _prep_core_inputs(
        A, S,
        np.asarray(W1, np.float32), np.asarray(b1, np.float32),
        np.asarray(W2, np.float32), np.asarray(b2, np.float32),
        np.asarray(W3, np.float32), np.asarray(b3, np.float32),
        np.asarray(W4, np.float32), np.asarray(b4, np.float32),
        np.asarray(Wl, np.float32), np.asarray(bl, np.float32),
    )
    nc = _get_program()
    res = run_bass_kernel_spmd(nc, in_maps, list(range(NCORES)))
    return np.concatenate([res.results[c]["OUT"] for c in range(NCORES)],
                          axis=0)
